# revision 11
# baseline (speedup 1.0000x reference)
"""Trainium2 Bass kernel for nn_ComputeVecLoss (vector loss over keypoint graphs).

Math (per batch b):
  For every keypoint pair (i>j) sample 5 points on the segment; cdis = mean
  over the 5 points of the min squared distance to the 4096 gt points; an edge
  exists when cdis < 1e-3.  Loss = sum over edges of |u_i.u_j| / (|u_i||u_j|)
  divided by (1 + edge count), u_k = p0 - p_k.

Design:
  * Each batch needs only 425 unique query points (17 endpoints + 136*3
    interiors).  Two batches per core -> 850 rows in 7 tiles of 128.
  * d2(r,m) = |K_r|^2 + |g_m|^2 - 2 K_r.g_m is ONE TensorEngine contraction
    of depth 8 handling both batches: kaugT rows [k2_b0, k2_b1, -2Kx0, -2Ky0,
    blk0, -2Kx1, -2Ky1, blk1] vs gaug rows [1, 1, gx0, gy0, g2_0-1/4, gx1,
    gy1, g2_1-1/4].  kaugT ([8,896]) is built on HOST (28KB DMA) so the PE
    starts as soon as the first gt chunk lands; gaug streams in 4 chunks of
    1024 columns pipelined with the first row-tile's matmuls.
  * min over m=4096 per row-tile: banks A,B hard-min on the Vector engine
    (tensor_reduce), banks C,D via exp-softmin on the Scalar engine (ACT Exp
    with accumulate); a small DVE decode merges the three partial mins.
    (A fused dual-input tensor_tensor_reduce evacuation faults on hardware.)
  * Row-tile 0 holds the 34 endpoints + 48 leftover-pair interiors, so the
    cosine epilog (selector gathers + chains) runs DURING the hot loop.
  * Sum-gather: s12 = s1+s2 gathers u_i+u_j / |u_i|^2+|u_j|^2 / absu_i+absu_j
    per pair in ONE matmul per group; dot and |u_i||u_j| come back via
    difference of squares (the 1/2 factors cancel in the ratio), and
    c3 = cdis-sum uses a TTR with initial_value = the gathered endpoint sums.

Sharding: batch dim 16 -> 8 cores x 2 batches.  Each core returns
[sum(cos), edge_count]; the host combines and divides.
"""

import os
import sys

for _p in ("/opt/trn_rl_repo",):
    if os.path.isdir(_p) and _p not in sys.path:
        sys.path.append(_p)

import numpy as np

B, N, D = 16, 17, 2
M = 4096
COUNT = 5
MAXDIS = 1e-3
EPS_ABS = 1e-5
N_CORES = 8
TSOFT = 8e-5           # softmin temperature
LNC = 34.657359028     # ln(2^50) prescale keeps es well inside fp32 normals
LN2 = 0.6931471805599453
BPC = B // N_CORES          # batches per core
NPAIR = N * (N - 1) // 2    # 136
RTILES = 7
RPAD = RTILES * 128         # 896
CONTR = 8                   # contraction depth of the hot matmul
PAIR2 = BPC * NPAIR         # 272 pairs per core
NLEFT = NPAIR - 128         # 8 leftover pairs per batch
SROWS = 2 * N + 2 * 3 * NLEFT  # 82 rows: 34 endpoints + 48 leftover triples
GROUPS = [(0, 128), (128, 128), (256, 2 * NLEFT)]
MCH = 4                     # gt DMA chunks
MCW = M // MCH              # 1024

PAIRS = [(i, j) for i in range(1, N) for j in range(i)]


def _row_endpoint(b, i):
    return N * b + i


def _row_leftover(b, q, k):
    return 2 * N + 3 * (NLEFT * b + q) + k


def _row_triple(b, p, k):
    # triples of the first 128 pairs: rtiles 1-3 are batch 0, 4-6 batch 1
    if p < 128:
        return 128 * (1 + k) + p if b == 0 else 128 * (4 + k) + p
    return _row_leftover(b, p - 128, k)


def _s12_const():
    s = np.zeros((SROWS, PAIR2), np.float32)
    for b in range(BPC):
        for p, (i, j) in enumerate(PAIRS):
            if p < 128:
                P = 128 * b + p
            else:
                P = 256 + NLEFT * b + (p - 128)
                for k in range(3):
                    s[_row_leftover(b, p - 128, k), P] = 1.0
            s[_row_endpoint(b, i), P] += 1.0
            s[_row_endpoint(b, j), P] += 1.0
    return s


_CONSTS = None
_COMPILED = None


def _get_consts():
    global _CONSTS
    if _CONSTS is None:
        _CONSTS = _s12_const()
    return _CONSTS


def _build():
    import concourse.bass as bass
    import concourse.bacc as bacc
    import concourse.tile as tile
    from concourse import mybir

    f32 = mybir.dt.float32
    f32r = mybir.dt.float32r
    bf16 = mybir.dt.bfloat16
    i32 = mybir.dt.int32
    Alu = mybir.AluOpType
    Act = mybir.ActivationFunctionType
    X = mybir.AxisListType.X

    nc = bacc.Bacc("TRN2", target_bir_lowering=False, debug=False,
                   num_devices=N_CORES)

    kaug_d = nc.dram_tensor("kaugT", [CONTR, RPAD], f32r,
                            kind="ExternalInput").ap()
    gtt_d = nc.dram_tensor("gtt", [CONTR, M], f32r, kind="ExternalInput").ap()
    s12_d = nc.dram_tensor("s12", [SROWS, PAIR2], f32,
                           kind="ExternalInput").ap()
    uex_d = nc.dram_tensor("uex", [SROWS, 5], f32, kind="ExternalInput").ap()
    out_d = nc.dram_tensor("out", [2], f32, kind="ExternalOutput").ap()

    with tile.TileContext(nc) as tc:
        with (
            tc.tile_pool(name="singles", bufs=1) as singles,
            tc.tile_pool(name="work", bufs=2) as work,
            tc.tile_pool(name="psum", bufs=3, space="PSUM") as psum,
            tc.tile_pool(name="psmall", bufs=2, space="PSUM") as psmall,
        ):
            # ---- input DMAs: both HWDGE rings, gt chunks pipelined --------
            gchunks = [singles.tile([CONTR, MCW], f32r, tag=f"g{c}",
                                    name=f"gchunk{c}") for c in range(MCH)]
            kaugT = singles.tile([CONTR, RPAD], f32r)
            uext6 = singles.tile([SROWS, 6], f32)
            s12_sb = singles.tile([SROWS, PAIR2], f32)

            nc.scalar.dma_start(out=gchunks[0][:],
                                in_=gtt_d[:, 0 * MCW:1 * MCW])
            nc.sync.dma_start(out=kaugT[:], in_=kaug_d[:])
            nc.sync.dma_start(out=gchunks[1][:], in_=gtt_d[:, 1 * MCW:2 * MCW])
            nc.scalar.dma_start(out=gchunks[2][:],
                                in_=gtt_d[:, 2 * MCW:3 * MCW])
            nc.sync.dma_start(out=gchunks[3][:], in_=gtt_d[:, 3 * MCW:4 * MCW])
            nc.scalar.dma_start(out=uext6[:, 0:5], in_=uex_d[:])
            nc.sync.dma_start(out=s12_sb[:], in_=s12_d[:])

            # ---- small setup: memsets on gpsimd, ACT table warm -----------
            onescol = singles.tile([128, 1], f32)
            nc.gpsimd.memset(onescol[:], 1.0)
            cm_all = singles.tile([128, 6], f32)
            nc.gpsimd.memset(cm_all[:], 0.0)
            warm = singles.tile([1, 2], f32)
            nc.gpsimd.memset(warm[:], 1.0)
            nc.scalar.activation(out=warm[:, 0:1], in_=warm[:, 1:2],
                                 func=Act.Square)
            nc.scalar.activation(out=warm[:, 0:1], in_=warm[:, 1:2],
                                 func=Act.Abs)
            nc.scalar.activation(out=warm[:, 0:1], in_=warm[:, 1:2],
                                 func=Act.Exp)
            lnc_sb = singles.tile([128, 1], f32)
            nc.gpsimd.memset(lnc_sb[:], float(LNC))

            pminA = singles.tile([128, RTILES], f32)
            pminB = singles.tile([128, RTILES], f32)
            pmin = singles.tile([128, RTILES], f32)
            es = singles.tile([128, RTILES], f32)
            soft = singles.tile([128, RTILES], f32)

            state = {}

            def emit_sel_mm(g):
                g0, cnt = GROUPS[g]
                pg = psmall.tile([128, 512], f32, tag="sm")
                nc.tensor.matmul(pg[0:cnt, 0:6], s12_sb[:, g0:g0 + cnt],
                                 uext6[:], start=True, stop=True)
                sb = work.tile([cnt, 6], f32, tag="sb" + str(g))
                nc.scalar.copy(out=sb[:], in_=pg[0:cnt, 0:6])
                state["sb" + str(g)] = sb

            def emit_chain_a(g):
                # cos = |u_i.u_j| / (|u_i||u_j|) via difference of squares
                g0, cnt = GROUPS[g]
                sb = state["sb" + str(g)]
                sq = work.tile([cnt, 3], f32, tag="sq" + str(g))
                dd = work.tile([cnt, 4], f32, tag="dd" + str(g))
                nc.scalar.activation(out=sq[:, 0:2], in_=sb[:, 0:2],
                                     func=Act.Square, accum_out=dd[:, 0:1])
                nc.scalar.activation(out=sq[:, 2:3], in_=sb[:, 3:4],
                                     func=Act.Square)
                nc.vector.tensor_tensor(out=dd[:, 1:2], in0=dd[:, 0:1],
                                        in1=sb[:, 2:3], op=Alu.subtract)
                nc.vector.tensor_tensor(out=dd[:, 2:3], in0=sq[:, 2:3],
                                        in1=sb[:, 4:5], op=Alu.subtract)
                rec = work.tile([cnt, 1], f32, tag="rc" + str(g))
                nc.vector.reciprocal(out=rec[:], in_=dd[:, 2:3])
                nc.scalar.activation(out=dd[:, 3:4], in_=dd[:, 1:2],
                                     func=Act.Abs)
                cosg = work.tile([cnt, 1], f32, tag="cs" + str(g))
                nc.scalar.activation(out=cosg[:], in_=dd[:, 3:4],
                                     func=Act.Copy, scale=rec[:])
                state["cos" + str(g)] = cosg

            def emit_chain_b(g):
                # c3 = sum_t pmin[triple cols] + gathered endpoint sums
                g0, cnt = GROUPS[g]
                sb = state["sb" + str(g)]
                if g < 2:
                    c0 = 1 + 3 * g
                    c3 = work.tile([cnt, 1], f32, tag="c3" + str(g))
                    nc.vector.tensor_reduce(out=c3[:],
                                            in_=pmin[0:cnt, c0:c0 + 3],
                                            axis=X, op=Alu.add)
                    nc.scalar.activation(out=c3[:], in_=c3[:],
                                         func=Act.Identity, bias=sb[:, 5:6])
                    c3ap = c3[:]
                else:
                    c3ap = sb[:, 5:6]
                msk = work.tile([cnt, 1], f32, tag="mk" + str(g))
                nc.vector.tensor_single_scalar(
                    out=msk[:], in_=c3ap, scalar=float(COUNT * MAXDIS),
                    op=Alu.is_lt)
                nc.scalar.copy(out=cm_all[0:cnt, 3 + g:4 + g], in_=msk[:])
                nc.scalar.activation(out=cm_all[0:cnt, g:g + 1],
                                     in_=state["cos" + str(g)],
                                     func=Act.Copy, scale=msk[:])

            # ---- hot loop: 7 row-tiles x 8 matmuls; banks A,B hard-min on
            #      DVE, banks C,D exp-softmin on ACT; small decode merges ---
            def finalize(t):
                eint = work.tile([128, 1], i32, tag="eint")
                ef = work.tile([128, 1], f32, tag="ef")
                nc.vector.tensor_single_scalar(
                    out=eint[:], in_=es[:, t:t + 1].bitcast(i32),
                    scalar=23, op=Alu.arith_shift_right)
                nc.vector.tensor_copy(out=ef[:], in_=eint[:])
                nc.vector.tensor_scalar(
                    out=soft[:, t:t + 1], in0=ef[:],
                    scalar1=float(-TSOFT * LN2),
                    scalar2=float(TSOFT * (127.0 * LN2 + LNC)),
                    op0=Alu.mult, op1=Alu.add)
                nc.vector.tensor_tensor(out=pmin[:, t:t + 1],
                                        in0=pminA[:, t:t + 1],
                                        in1=pminB[:, t:t + 1], op=Alu.min)
                nc.vector.tensor_tensor(out=pmin[:, t:t + 1],
                                        in0=pmin[:, t:t + 1],
                                        in1=soft[:, t:t + 1], op=Alu.min)

            for t in range(RTILES):
                wtile = kaugT[:, 128 * t:128 * (t + 1)]
                ps = []
                for h in range(4):
                    ph = psum.tile([128, 1024], f32, tag="hot")
                    for j in range(2):
                        nc.tensor.matmul(
                            ph[:, 512 * j:512 * (j + 1)], wtile,
                            gchunks[h][:, 512 * j:512 * (j + 1)],
                            start=True, stop=True)
                    ps.append(ph)
                    if h == 0:
                        nc.vector.tensor_reduce(out=pminA[:, t:t + 1],
                                                in_=ps[0][:], axis=X,
                                                op=Alu.min)
                    if h == 1:
                        nc.vector.tensor_reduce(out=pminB[:, t:t + 1],
                                                in_=ps[1][:], axis=X,
                                                op=Alu.min)
                    if h == 2:
                        junkC = work.tile([128, 1024], bf16, tag="jC")
                        eC = work.tile([128, 1], f32, tag="eC")
                        nc.scalar.activation(out=junkC[:], in_=ps[2][:],
                                             func=Act.Exp,
                                             scale=float(-1.0 / TSOFT),
                                             bias=lnc_sb[:], accum_out=eC[:])
                junkD = work.tile([128, 1024], bf16, tag="jD")
                eD = work.tile([128, 1], f32, tag="eD")
                nc.scalar.activation(out=junkD[:], in_=ps[3][:], func=Act.Exp,
                                     scale=float(-1.0 / TSOFT),
                                     bias=lnc_sb[:], accum_out=eD[:])
                nc.gpsimd.tensor_add(es[:, t:t + 1], eC[:], eD[:])
                if t > 0:
                    finalize(t - 1)

                if t == 1:
                    # endpoint/leftover mins -> uext6 col 5
                    nc.vector.tensor_copy(out=uext6[0:SROWS, 5:6],
                                          in_=pmin[0:SROWS, 0:1])
                    emit_sel_mm(0)
                    emit_sel_mm(1)
                    emit_sel_mm(2)
                if t == 2:
                    emit_chain_a(0)
                    emit_chain_a(1)
                    emit_chain_a(2)
                    emit_chain_b(2)
                if t == 5:
                    emit_chain_b(0)

            finalize(RTILES - 1)
            emit_chain_b(1)

            tot = psmall.tile([128, 512], f32, tag="sm")
            nc.tensor.matmul(tot[0:1, 0:6], onescol[:], cm_all[:],
                             start=True, stop=True)
            res = work.tile([1, 2], f32, tag="res")
            tot3 = tot[0:1, 0:6].rearrange("p (j c) -> p j c", c=3)
            nc.vector.tensor_reduce(out=res[:], in_=tot3, axis=X, op=Alu.add)
            nc.sync.dma_start(out=out_d.rearrange("(a b) -> a b", a=1),
                              in_=res[:])

    nc.compile()
    return nc


def _make_in_maps(recon_points: np.ndarray, gt_points: np.ndarray):
    s12 = _get_consts()
    recon_points = np.ascontiguousarray(recon_points, np.float32)
    gt_points = np.ascontiguousarray(gt_points, np.float32)

    t_int = np.array([0.25, 0.5, 0.75], np.float32)          # interior t
    pi = np.array([p[0] for p in PAIRS], np.int32)
    pj = np.array([p[1] for p in PAIRS], np.int32)

    in_maps = []
    for k in range(N_CORES):
        rec = recon_points[BPC * k:BPC * (k + 1)]            # [2, 17, 2]
        # query points K for every row of the permuted layout
        K = np.zeros((RPAD, D), np.float32)
        blk = np.zeros((RPAD, BPC), np.float32)
        for b in range(BPC):
            P1 = rec[b]                                      # [17, 2]
            K[N * b:N * b + N] = P1                          # endpoints
            blk[N * b:N * b + N, b] = 1.0
            # interiors of pairs 0..127 -> rtiles 1-3 (b0) / 4-6 (b1)
            seg = (t_int[:, None, None] * P1[pi[:128]][None]
                   + (1.0 - t_int)[:, None, None] * P1[pj[:128]][None])
            base = 128 * (1 + 3 * b)
            K[base:base + 384] = seg.reshape(384, 2)
            blk[base:base + 384, b] = 1.0
            # leftover pair interiors -> rows 34..81 of rtile 0
            segl = (t_int[None, :, None] * P1[pi[128:]][:, None]
                    + (1.0 - t_int)[None, :, None] * P1[pj[128:]][:, None])
            lbase = 2 * N + 3 * NLEFT * b
            K[lbase:lbase + 3 * NLEFT] = segl.reshape(3 * NLEFT, 2)
            blk[lbase:lbase + 3 * NLEFT, b] = 1.0
        k2 = (K * K).sum(-1) + 0.25
        kaugT = np.zeros((CONTR, RPAD), np.float32)
        kaugT[0] = k2 * blk[:, 0]
        kaugT[1] = k2 * blk[:, 1]
        for b in range(BPC):
            kaugT[2 + 3 * b] = -2.0 * K[:, 0] * blk[:, b]
            kaugT[3 + 3 * b] = -2.0 * K[:, 1] * blk[:, b]
            kaugT[4 + 3 * b] = blk[:, b]

        gt_pair = gt_points[BPC * k:BPC * (k + 1)]           # [2, 4096, 2]
        gtt = np.empty((CONTR, M), np.float32)
        gtt[0:2] = 1.0
        gtt[2:4] = gt_pair[0].T
        gtt[4] = (gt_pair[0] * gt_pair[0]).sum(-1) - 0.25
        gtt[5:7] = gt_pair[1].T
        gtt[7] = (gt_pair[1] * gt_pair[1]).sum(-1) - 0.25

        uex = np.zeros((SROWS, 5), np.float32)
        for b in range(BPC):
            u = rec[b, 0:1] - rec[b]                         # [17, 2]
            u2 = (u * u).sum(-1)
            uex[N * b:N * b + N, 0:2] = u
            uex[N * b:N * b + N, 2] = u2
            uex[N * b:N * b + N, 4] = u2 + D * EPS_ABS
            uex[N * b:N * b + N, 3] = np.sqrt(u2 + D * EPS_ABS)

        in_maps.append({"kaugT": kaugT, "gtt": gtt, "s12": s12, "uex": uex})
    return in_maps


def kernel(recon_points: np.ndarray, gt_points: np.ndarray) -> np.ndarray:
    from concourse.bass_utils import run_bass_kernel_spmd

    global _COMPILED
    if _COMPILED is None:
        _COMPILED = _build()
    nc = _COMPILED

    in_maps = _make_in_maps(recon_points, gt_points)
    res = run_bass_kernel_spmd(nc, in_maps, core_ids=list(range(N_CORES)))
    partials = np.stack([r["out"] for r in res.results])  # [8, 2]
    cos_sum = partials[:, 0].sum(dtype=np.float32)
    cnt = partials[:, 1].sum(dtype=np.float32)
    return np.float32(cos_sum / (np.float32(1.0) + cnt))


# revision 12
# speedup vs baseline: 1.0945x; 1.0945x over previous
"""Trainium2 Bass kernel for nn_ComputeVecLoss (vector loss over keypoint graphs).

Math (per batch b):
  For every keypoint pair (i>j) sample 5 points on the segment; cdis = mean
  over the 5 points of the min squared distance to the 4096 gt points; an edge
  exists when cdis < 1e-3.  Loss = sum over edges of |u_i.u_j| / (|u_i||u_j|)
  divided by (1 + edge count), u_k = p0 - p_k.

Design:
  * Each batch needs only 425 unique query points (17 endpoints + 136*3
    interiors).  Two batches per core -> 850 rows in 7 tiles of 128.
  * d2(r,m) = |K_r|^2 + |g_m|^2 - 2 K_r.g_m is ONE TensorEngine contraction
    of depth 8 handling both batches: kaugT rows [k2_b0, k2_b1, -2Kx0, -2Ky0,
    blk0, -2Kx1, -2Ky1, blk1] vs gaug rows [1, 1, gx0, gy0, g2_0-1/4, gx1,
    gy1, g2_1-1/4].  kaugT ([8,896]) is built on HOST and ships in the same
    DMA as the first half of the gt rows, so the PE starts ~3us in.
  * min over m=4096 per row-tile: banks A,B (m<2048) via exp-softmin on the
    Scalar engine (ACT Exp with accumulate - frees their PSUM banks early),
    banks C,D hard-min tensor_reduce on Vector.  The softmin exponent decode
    and the 3-way min run on the otherwise-idle GpSimd engine.
  * Row-tile 0 holds the 34 endpoints + 48 leftover-pair interiors, so the
    cosine epilog (selector gathers + chains) runs DURING the hot loop;
    only the last group's threshold test and the final sum trail the loop.
  * Sum-gather: s12 = s1+s2 gathers u_i+u_j / |u_i|^2+|u_j|^2 / absu_i+absu_j
    per pair in ONE matmul per group; dot and |u_i||u_j| come back via
    difference of squares (the 1/2 factors cancel in the ratio).

Sharding: batch dim 16 -> 8 cores x 2 batches.  Each core returns
[sum(cos), edge_count]; the host combines and divides.
"""

import os
import sys

for _p in ("/opt/trn_rl_repo",):
    if os.path.isdir(_p) and _p not in sys.path:
        sys.path.append(_p)

import numpy as np

B, N, D = 16, 17, 2
M = 4096
COUNT = 5
MAXDIS = 1e-3
EPS_ABS = 1e-5
N_CORES = 8
TSOFT = 8e-5           # softmin temperature
LNC = 34.657359028     # ln(2^50) prescale keeps es well inside fp32 normals
LN2 = 0.6931471805599453
BPC = B // N_CORES          # batches per core
NPAIR = N * (N - 1) // 2    # 136
RTILES = 7
RPAD = RTILES * 128         # 896
CONTR = 8                   # contraction depth of the hot matmul
PAIR2 = BPC * NPAIR         # 272 pairs per core
NLEFT = NPAIR - 128         # 8 leftover pairs per batch
SROWS = 2 * N + 2 * 3 * NLEFT  # 82 rows: 34 endpoints + 48 leftover triples
GROUPS = [(0, 128), (128, 128), (256, 2 * NLEFT)]
KG1W = RPAD + M // 2        # kaugT + first gt half
KG2W = M // 2
SUW = PAIR2 + 5             # s12 + uex columns

PAIRS = [(i, j) for i in range(1, N) for j in range(i)]


def _row_endpoint(b, i):
    return N * b + i


def _row_leftover(b, q, k):
    return 2 * N + 3 * (NLEFT * b + q) + k


def _s12_const():
    s = np.zeros((SROWS, PAIR2), np.float32)
    for b in range(BPC):
        for p, (i, j) in enumerate(PAIRS):
            if p < 128:
                P = 128 * b + p
            else:
                P = 256 + NLEFT * b + (p - 128)
                for k in range(3):
                    s[_row_leftover(b, p - 128, k), P] = 1.0
            s[_row_endpoint(b, i), P] += 1.0
            s[_row_endpoint(b, j), P] += 1.0
    return s


_CONSTS = None
_COMPILED = None


def _get_consts():
    global _CONSTS
    if _CONSTS is None:
        _CONSTS = _s12_const()
    return _CONSTS


def _build():
    import concourse.bass as bass
    import concourse.bacc as bacc
    import concourse.tile as tile
    from concourse import mybir

    f32 = mybir.dt.float32
    f32r = mybir.dt.float32r
    bf16 = mybir.dt.bfloat16
    i32 = mybir.dt.int32
    Alu = mybir.AluOpType
    Act = mybir.ActivationFunctionType
    X = mybir.AxisListType.X

    nc = bacc.Bacc("TRN2", target_bir_lowering=False, debug=False,
                   num_devices=N_CORES)

    kg1_d = nc.dram_tensor("kg1", [CONTR, KG1W], f32r,
                           kind="ExternalInput").ap()
    kg2_d = nc.dram_tensor("kg2", [CONTR, KG2W], f32r,
                           kind="ExternalInput").ap()
    su_d = nc.dram_tensor("su", [SROWS, SUW], f32, kind="ExternalInput").ap()
    out_d = nc.dram_tensor("out", [2], f32, kind="ExternalOutput").ap()

    with tile.TileContext(nc) as tc:
        with (
            tc.tile_pool(name="singles", bufs=1) as singles,
            tc.tile_pool(name="work", bufs=2) as work,
            tc.tile_pool(name="psum", bufs=1, space="PSUM") as psum,
        ):
            # ---- inputs: one transfer per ring, epilog constants second ---
            kg1 = singles.tile([CONTR, KG1W], f32r)
            kg2 = singles.tile([CONTR, KG2W], f32r)
            su = singles.tile([SROWS, SUW], f32)
            nc.sync.dma_start(out=kg1[:], in_=kg1_d[:])
            nc.scalar.dma_start(out=kg2[:], in_=kg2_d[:])
            nc.sync.dma_start(out=su[:], in_=su_d[:])
            s12_sb = su[:, 0:PAIR2]
            mslice = [kg1[:, RPAD:RPAD + 1024],
                      kg1[:, RPAD + 1024:RPAD + 2048],
                      kg2[:, 0:1024], kg2[:, 1024:2048]]

            # ---- small setup: memsets on gpsimd, ACT table warm -----------
            onescol = singles.tile([128, 1], f32)
            nc.gpsimd.memset(onescol[:], 1.0)
            cm_all = singles.tile([128, 6], f32)
            nc.gpsimd.memset(cm_all[:], 0.0)
            lnc_sb = singles.tile([128, 1], f32)
            nc.gpsimd.memset(lnc_sb[:], float(LNC))
            warm = singles.tile([1, 2], f32)
            nc.gpsimd.memset(warm[:], 1.0)
            nc.scalar.activation(out=warm[:, 0:1], in_=warm[:, 1:2],
                                 func=Act.Square)
            nc.scalar.activation(out=warm[:, 0:1], in_=warm[:, 1:2],
                                 func=Act.Abs)
            nc.scalar.activation(out=warm[:, 0:1], in_=warm[:, 1:2],
                                 func=Act.Exp)

            uext6 = singles.tile([SROWS, 6], f32)
            pminC = singles.tile([128, RTILES], f32)
            pminD = singles.tile([128, RTILES], f32)
            pmin = singles.tile([128, RTILES], f32)
            es = singles.tile([128, RTILES], f32)
            soft = singles.tile([128, RTILES], f32)

            state = {}

            def emit_sel_mm(g):
                g0, cnt = GROUPS[g]
                pg = psum.tile([cnt, 6], f32, tag="A", name=f"pg{g}")
                nc.tensor.matmul(pg[:], s12_sb[:, g0:g0 + cnt],
                                 uext6[:], start=True, stop=True)
                sb = work.tile([cnt, 6], f32, tag="sb" + str(g),
                               name=f"sbg{g}")
                nc.scalar.copy(out=sb[:], in_=pg[:])
                state["sb" + str(g)] = sb

            def emit_chain_a(g):
                # cos = |u_i.u_j| / (|u_i||u_j|) via difference of squares
                g0, cnt = GROUPS[g]
                sb = state["sb" + str(g)]
                sq = work.tile([cnt, 3], f32, tag="sq" + str(g),
                               name=f"sq{g}")
                dd = work.tile([cnt, 4], f32, tag="dd" + str(g),
                               name=f"dd{g}")
                nc.scalar.activation(out=sq[:, 0:2], in_=sb[:, 0:2],
                                     func=Act.Square, accum_out=dd[:, 0:1])
                nc.scalar.activation(out=sq[:, 2:3], in_=sb[:, 3:4],
                                     func=Act.Square)
                nc.vector.tensor_tensor(out=dd[:, 1:2], in0=dd[:, 0:1],
                                        in1=sb[:, 2:3], op=Alu.subtract)
                nc.vector.tensor_tensor(out=dd[:, 2:3], in0=sq[:, 2:3],
                                        in1=sb[:, 4:5], op=Alu.subtract)
                rec = work.tile([cnt, 1], f32, tag="rc" + str(g),
                                name=f"rc{g}")
                nc.vector.reciprocal(out=rec[:], in_=dd[:, 2:3])
                nc.scalar.activation(out=dd[:, 3:4], in_=dd[:, 1:2],
                                     func=Act.Abs)
                cosg = work.tile([cnt, 1], f32, tag="cs" + str(g),
                                 name=f"cs{g}")
                nc.scalar.activation(out=cosg[:], in_=dd[:, 3:4],
                                     func=Act.Copy, scale=rec[:])
                state["cos" + str(g)] = cosg

            def emit_mask_cm(g, c3ap):
                g0, cnt = GROUPS[g]
                msk = work.tile([cnt, 1], f32, tag="mk" + str(g),
                                name=f"mk{g}")
                nc.vector.tensor_single_scalar(
                    out=msk[:], in_=c3ap, scalar=float(COUNT * MAXDIS),
                    op=Alu.is_lt)
                nc.scalar.copy(out=cm_all[0:cnt, 3 + g:4 + g], in_=msk[:])
                nc.scalar.activation(out=cm_all[0:cnt, g:g + 1],
                                     in_=state["cos" + str(g)],
                                     func=Act.Copy, scale=msk[:])

            def emit_chain_b(g):
                g0, cnt = GROUPS[g]
                sb = state["sb" + str(g)]
                if g == 2:
                    emit_mask_cm(2, sb[:, 5:6])
                    return
                c0 = 1 + 3 * g
                c3 = work.tile([cnt, 1], f32, tag="c3" + str(g),
                               name=f"c3{g}")
                nc.vector.tensor_reduce(out=c3[:],
                                        in_=pmin[0:cnt, c0:c0 + 3],
                                        axis=X, op=Alu.add)
                nc.scalar.activation(out=c3[:], in_=c3[:],
                                     func=Act.Identity, bias=sb[:, 5:6])
                emit_mask_cm(g, c3[:])

            # softmin decode + 3-way min on GpSimd (idle engine)
            def finalize(t):
                eint = work.tile([128, 1], i32, tag="eint")
                ef = work.tile([128, 1], f32, tag="ef")
                nc.gpsimd.tensor_single_scalar(
                    out=eint[:], in_=es[:, t:t + 1].bitcast(i32),
                    scalar=23, op=Alu.arith_shift_right)
                nc.gpsimd.tensor_copy(out=ef[:], in_=eint[:])
                nc.gpsimd.tensor_scalar(
                    out=soft[:, t:t + 1], in0=ef[:],
                    scalar1=float(-TSOFT * LN2),
                    scalar2=float(TSOFT * (127.0 * LN2 + LNC)),
                    op0=Alu.mult, op1=Alu.add)
                nc.gpsimd.tensor_tensor(out=pmin[:, t:t + 1],
                                        in0=pminC[:, t:t + 1],
                                        in1=pminD[:, t:t + 1], op=Alu.min)
                nc.gpsimd.tensor_tensor(out=pmin[:, t:t + 1],
                                        in0=pmin[:, t:t + 1],
                                        in1=soft[:, t:t + 1], op=Alu.min)

            # ---- hot loop: 7 row-tiles x 8 matmuls ------------------------
            TAGS = ("A", "B", "C", "D")
            for t in range(RTILES):
                wtile = kg1[:, 128 * t:128 * (t + 1)]
                for h in range(4):
                    ph = psum.tile([128, 1024], f32, tag=TAGS[h],
                                   name=f"p{TAGS[h]}")
                    for j in range(2):
                        nc.tensor.matmul(
                            ph[:, 512 * j:512 * (j + 1)], wtile,
                            mslice[h][:, 512 * j:512 * (j + 1)],
                            start=True, stop=True)
                    if h == 0:
                        # softmin bank A; Exp frees the bank early so the
                        # selector matmuls can slot into the A ring
                        junkA = work.tile([128, 1024], bf16, tag="jA")
                        eA = work.tile([128, 1], f32, tag="eA")
                        nc.scalar.activation(out=junkA[:], in_=ph[:],
                                             func=Act.Exp,
                                             scale=float(-1.0 / TSOFT),
                                             bias=lnc_sb[:], accum_out=eA[:])
                        if 2 <= t <= 4:
                            emit_sel_mm(t - 2)
                    elif h == 1:
                        junkB = work.tile([128, 1024], bf16, tag="jB")
                        eB = work.tile([128, 1], f32, tag="eB")
                        nc.scalar.activation(out=junkB[:], in_=ph[:],
                                             func=Act.Exp,
                                             scale=float(-1.0 / TSOFT),
                                             bias=lnc_sb[:], accum_out=eB[:])
                        nc.gpsimd.tensor_add(es[:, t:t + 1], eA[:], eB[:])
                    elif h == 2:
                        nc.vector.tensor_reduce(out=pminC[:, t:t + 1],
                                                in_=ph[:], axis=X, op=Alu.min)
                    else:
                        nc.vector.tensor_reduce(out=pminD[:, t:t + 1],
                                                in_=ph[:], axis=X, op=Alu.min)

                if t > 0:
                    finalize(t - 1)
                if t == 1:
                    # endpoint/leftover mins -> uext6 (epilog gather moving)
                    nc.vector.tensor_copy(out=uext6[:, 0:5],
                                          in_=su[:, PAIR2:PAIR2 + 5])
                    nc.vector.tensor_copy(out=uext6[0:SROWS, 5:6],
                                          in_=pmin[0:SROWS, 0:1])
                if t == 3:
                    emit_chain_a(0)
                if t == 4:
                    emit_chain_a(1)
                if t == 5:
                    emit_chain_a(2)
                    emit_chain_b(2)
                    emit_chain_b(0)
                if t == 6:
                    # partial c3 for the last group: cols 4,5 + gathered sums
                    sb1 = state["sb1"]
                    c3p = work.tile([128, 1], f32, tag="c3p")
                    nc.vector.tensor_reduce(out=c3p[:],
                                            in_=pmin[0:128, 4:6],
                                            axis=X, op=Alu.add)
                    nc.scalar.activation(out=c3p[:], in_=c3p[:],
                                         func=Act.Identity, bias=sb1[:, 5:6])
                    state["c3p"] = c3p

            finalize(RTILES - 1)
            c3f = work.tile([128, 1], f32, tag="c3f")
            nc.vector.tensor_tensor(out=c3f[:], in0=state["c3p"][:],
                                    in1=pmin[0:128, 6:7], op=Alu.add)
            emit_mask_cm(1, c3f[:])

            tot = psum.tile([1, 6], f32, tag="A")
            nc.tensor.matmul(tot[:], onescol[:], cm_all[:],
                             start=True, stop=True)
            res = work.tile([1, 2], f32, tag="res")
            tot3 = tot[:].rearrange("p (j c) -> p j c", c=3)
            nc.vector.tensor_reduce(out=res[:], in_=tot3, axis=X, op=Alu.add)
            nc.sync.dma_start(out=out_d.rearrange("(a b) -> a b", a=1),
                              in_=res[:])

    nc.compile()
    return nc


def _make_in_maps(recon_points: np.ndarray, gt_points: np.ndarray):
    s12 = _get_consts()
    recon_points = np.ascontiguousarray(recon_points, np.float32)
    gt_points = np.ascontiguousarray(gt_points, np.float32)

    t_int = np.array([0.25, 0.5, 0.75], np.float32)          # interior t
    pi = np.array([p[0] for p in PAIRS], np.int32)
    pj = np.array([p[1] for p in PAIRS], np.int32)

    in_maps = []
    for k in range(N_CORES):
        rec = recon_points[BPC * k:BPC * (k + 1)]            # [2, 17, 2]
        # query points K for every row of the permuted layout
        K = np.zeros((RPAD, D), np.float32)
        blk = np.zeros((RPAD, BPC), np.float32)
        for b in range(BPC):
            P1 = rec[b]                                      # [17, 2]
            K[N * b:N * b + N] = P1                          # endpoints
            blk[N * b:N * b + N, b] = 1.0
            # interiors of pairs 0..127 -> rtiles 1-3 (b0) / 4-6 (b1)
            seg = (t_int[:, None, None] * P1[pi[:128]][None]
                   + (1.0 - t_int)[:, None, None] * P1[pj[:128]][None])
            base = 128 * (1 + 3 * b)
            K[base:base + 384] = seg.reshape(384, 2)
            blk[base:base + 384, b] = 1.0
            # leftover pair interiors -> rows 34..81 of rtile 0
            segl = (t_int[None, :, None] * P1[pi[128:]][:, None]
                    + (1.0 - t_int)[None, :, None] * P1[pj[128:]][:, None])
            lbase = 2 * N + 3 * NLEFT * b
            K[lbase:lbase + 3 * NLEFT] = segl.reshape(3 * NLEFT, 2)
            blk[lbase:lbase + 3 * NLEFT, b] = 1.0
        k2 = (K * K).sum(-1) + 0.25
        kaugT = np.zeros((CONTR, RPAD), np.float32)
        kaugT[0] = k2 * blk[:, 0]
        kaugT[1] = k2 * blk[:, 1]
        for b in range(BPC):
            kaugT[2 + 3 * b] = -2.0 * K[:, 0] * blk[:, b]
            kaugT[3 + 3 * b] = -2.0 * K[:, 1] * blk[:, b]
            kaugT[4 + 3 * b] = blk[:, b]

        gt_pair = gt_points[BPC * k:BPC * (k + 1)]           # [2, 4096, 2]
        gtt = np.empty((CONTR, M), np.float32)
        gtt[0:2] = 1.0
        gtt[2:4] = gt_pair[0].T
        gtt[4] = (gt_pair[0] * gt_pair[0]).sum(-1) - 0.25
        gtt[5:7] = gt_pair[1].T
        gtt[7] = (gt_pair[1] * gt_pair[1]).sum(-1) - 0.25

        uex = np.zeros((SROWS, 5), np.float32)
        for b in range(BPC):
            u = rec[b, 0:1] - rec[b]                         # [17, 2]
            u2 = (u * u).sum(-1)
            uex[N * b:N * b + N, 0:2] = u
            uex[N * b:N * b + N, 2] = u2
            uex[N * b:N * b + N, 4] = u2 + D * EPS_ABS
            uex[N * b:N * b + N, 3] = np.sqrt(u2 + D * EPS_ABS)

        kg1 = np.concatenate([kaugT, gtt[:, :M // 2]], 1)
        kg2 = np.ascontiguousarray(gtt[:, M // 2:])
        su = np.concatenate([s12, uex], 1)
        in_maps.append({"kg1": kg1, "kg2": kg2, "su": su})
    return in_maps


def kernel(recon_points: np.ndarray, gt_points: np.ndarray) -> np.ndarray:
    from concourse.bass_utils import run_bass_kernel_spmd

    global _COMPILED
    if _COMPILED is None:
        _COMPILED = _build()
    nc = _COMPILED

    in_maps = _make_in_maps(recon_points, gt_points)
    res = run_bass_kernel_spmd(nc, in_maps, core_ids=list(range(N_CORES)))
    partials = np.stack([r["out"] for r in res.results])  # [8, 2]
    cos_sum = partials[:, 0].sum(dtype=np.float32)
    cnt = partials[:, 1].sum(dtype=np.float32)
    return np.float32(cos_sum / (np.float32(1.0) + cnt))


# revision 13
# speedup vs baseline: 1.1284x; 1.0310x over previous
"""Trainium2 Bass kernel for nn_ComputeVecLoss (vector loss over keypoint graphs).

Math (per batch b):
  For every keypoint pair (i>j) sample 5 points on the segment; cdis = mean
  over the 5 points of the min squared distance to the 4096 gt points; an edge
  exists when cdis < 1e-3.  Loss = sum over edges of |u_i.u_j| / (|u_i||u_j|)
  divided by (1 + edge count), u_k = p0 - p_k.

Design:
  * Each batch needs only 425 unique query points (17 endpoints + 136*3
    interiors).  Two batches per core -> 850 rows in 7 tiles of 128.
  * d2(r,m) = |K_r|^2 + |g_m|^2 - 2 K_r.g_m is ONE TensorEngine contraction
    of depth 8 handling both batches: kaugT rows [k2_b0, k2_b1, -2Kx0, -2Ky0,
    blk0, -2Kx1, -2Ky1, blk1] vs gaug rows [1, 1, gx0, gy0, g2_0-1/4, gx1,
    gy1, g2_1-1/4].  kaugT ([8,896]) is built on HOST and ships in the same
    DMA as the first half of the gt rows, so the PE starts ~3us in.
  * min over m=4096 per row-tile: banks A,B (m<2048) via exp-softmin on the
    Scalar engine (ACT Exp with accumulate - frees their PSUM banks early),
    banks C,D hard-min tensor_reduce on Vector.  The softmin exponent decode
    and the 3-way min run on the otherwise-idle GpSimd engine.
  * Row-tile 0 holds the 34 endpoints + 48 leftover-pair interiors, so the
    cosine epilog (selector gathers + chains) runs DURING the hot loop;
    only the last group's threshold test and the final sum trail the loop.
  * Sum-gather: s12 = s1+s2 gathers u_i+u_j / |u_i|^2+|u_j|^2 / absu_i+absu_j
    per pair in ONE matmul per group; dot and |u_i||u_j| come back via
    difference of squares (the 1/2 factors cancel in the ratio).

Sharding: batch dim 16 -> 8 cores x 2 batches.  Each core returns
[sum(cos), edge_count]; the host combines and divides.
"""

import os
import sys

for _p in ("/opt/trn_rl_repo",):
    if os.path.isdir(_p) and _p not in sys.path:
        sys.path.append(_p)

import numpy as np

B, N, D = 16, 17, 2
M = 4096
COUNT = 5
MAXDIS = 1e-3
EPS_ABS = 1e-5
N_CORES = 8
TSOFT = 8e-5           # softmin temperature
LNC = 34.657359028     # ln(2^50) prescale keeps es well inside fp32 normals
LN2 = 0.6931471805599453
BPC = B // N_CORES          # batches per core
NPAIR = N * (N - 1) // 2    # 136
RTILES = 7
RPAD = RTILES * 128         # 896
CONTR = 8                   # contraction depth of the hot matmul
PAIR2 = BPC * NPAIR         # 272 pairs per core
NLEFT = NPAIR - 128         # 8 leftover pairs per batch
SROWS = 2 * N + 2 * 3 * NLEFT  # 82 rows: 34 endpoints + 48 leftover triples
GROUPS = [(0, 128), (128, 128), (256, 2 * NLEFT)]
KG1W = RPAD + M // 2        # kaugT + first gt half
KG2W = M // 2
SUW = PAIR2 + 5             # s12 + uex columns

PAIRS = [(i, j) for i in range(1, N) for j in range(i)]


def _row_endpoint(b, i):
    return N * b + i


def _row_leftover(b, q, k):
    return 2 * N + 3 * (NLEFT * b + q) + k


def _s12_const():
    s = np.zeros((SROWS, PAIR2), np.float32)
    for b in range(BPC):
        for p, (i, j) in enumerate(PAIRS):
            if p < 128:
                P = 128 * b + p
            else:
                P = 256 + NLEFT * b + (p - 128)
                for k in range(3):
                    s[_row_leftover(b, p - 128, k), P] = 1.0
            s[_row_endpoint(b, i), P] += 1.0
            s[_row_endpoint(b, j), P] += 1.0
    return s


_CONSTS = None
_COMPILED = None


def _get_consts():
    global _CONSTS
    if _CONSTS is None:
        _CONSTS = _s12_const()
    return _CONSTS


def _build():
    import concourse.bass as bass
    import concourse.bacc as bacc
    import concourse.tile as tile
    from concourse import mybir

    f32 = mybir.dt.float32
    f32r = mybir.dt.float32r
    bf16 = mybir.dt.bfloat16
    i32 = mybir.dt.int32
    Alu = mybir.AluOpType
    Act = mybir.ActivationFunctionType
    X = mybir.AxisListType.X

    nc = bacc.Bacc("TRN2", target_bir_lowering=False, debug=False,
                   num_devices=N_CORES)

    kg1_d = nc.dram_tensor("kg1", [CONTR, KG1W], f32r,
                           kind="ExternalInput").ap()
    kg2_d = nc.dram_tensor("kg2", [CONTR, KG2W], f32r,
                           kind="ExternalInput").ap()
    su_d = nc.dram_tensor("su", [SROWS, SUW], f32, kind="ExternalInput").ap()
    out_d = nc.dram_tensor("out", [2], f32, kind="ExternalOutput").ap()

    with tile.TileContext(nc) as tc:
        with (
            tc.tile_pool(name="singles", bufs=1) as singles,
            tc.tile_pool(name="work", bufs=2) as work,
            tc.tile_pool(name="psum", bufs=1, space="PSUM") as psum,
        ):
            # ---- inputs: one transfer per ring, epilog constants second ---
            kg1 = singles.tile([CONTR, KG1W], f32r)
            kg2 = singles.tile([CONTR, KG2W], f32r)
            su = singles.tile([SROWS, SUW], f32)
            nc.sync.dma_start(out=kg1[:], in_=kg1_d[:])
            nc.scalar.dma_start(out=kg2[:], in_=kg2_d[:])
            nc.sync.dma_start(out=su[:], in_=su_d[:])
            s12_sb = su[:, 0:PAIR2]
            mslice = [kg1[:, RPAD:RPAD + 1024],
                      kg1[:, RPAD + 1024:RPAD + 2048],
                      kg2[:, 0:1024], kg2[:, 1024:2048]]

            # ---- small setup: memsets on gpsimd, ACT table warm -----------
            onescol = singles.tile([128, 1], f32)
            nc.gpsimd.memset(onescol[:], 1.0)
            cm_all = singles.tile([128, 6], f32)
            nc.gpsimd.memset(cm_all[:], 0.0)
            lnc_sb = singles.tile([128, 1], f32)
            nc.gpsimd.memset(lnc_sb[:], float(LNC))
            warm = singles.tile([1, 2], f32)
            nc.gpsimd.memset(warm[:], 1.0)
            nc.scalar.activation(out=warm[:, 0:1], in_=warm[:, 1:2],
                                 func=Act.Square)
            nc.scalar.activation(out=warm[:, 0:1], in_=warm[:, 1:2],
                                 func=Act.Abs)
            nc.scalar.activation(out=warm[:, 0:1], in_=warm[:, 1:2],
                                 func=Act.Exp)

            uext6 = singles.tile([SROWS, 6], f32)
            pminC = singles.tile([128, RTILES], f32)
            pminD = singles.tile([128, RTILES], f32)
            pmin = singles.tile([128, RTILES], f32)
            es = singles.tile([128, RTILES], f32)
            soft = singles.tile([128, RTILES], f32)

            state = {}

            def emit_sel_mm(g):
                g0, cnt = GROUPS[g]
                pg = psum.tile([cnt, 6], f32, tag="A", name=f"pg{g}")
                nc.tensor.matmul(pg[:], s12_sb[:, g0:g0 + cnt],
                                 uext6[:], start=True, stop=True)
                sb = work.tile([cnt, 6], f32, tag="sb" + str(g),
                               name=f"sbg{g}")
                nc.scalar.copy(out=sb[:], in_=pg[:])
                state["sb" + str(g)] = sb

            def emit_chain_a(g):
                # cos = |u_i.u_j| / (|u_i||u_j|) via difference of squares
                g0, cnt = GROUPS[g]
                sb = state["sb" + str(g)]
                sq = work.tile([cnt, 3], f32, tag="sq" + str(g),
                               name=f"sq{g}")
                dd = work.tile([cnt, 4], f32, tag="dd" + str(g),
                               name=f"dd{g}")
                nc.scalar.activation(out=sq[:, 0:2], in_=sb[:, 0:2],
                                     func=Act.Square, accum_out=dd[:, 0:1])
                nc.scalar.activation(out=sq[:, 2:3], in_=sb[:, 3:4],
                                     func=Act.Square)
                nc.vector.tensor_tensor(out=dd[:, 1:2], in0=dd[:, 0:1],
                                        in1=sb[:, 2:3], op=Alu.subtract)
                nc.vector.tensor_tensor(out=dd[:, 2:3], in0=sq[:, 2:3],
                                        in1=sb[:, 4:5], op=Alu.subtract)
                rec = work.tile([cnt, 1], f32, tag="rc" + str(g),
                                name=f"rc{g}")
                nc.vector.reciprocal(out=rec[:], in_=dd[:, 2:3])
                nc.scalar.activation(out=dd[:, 3:4], in_=dd[:, 1:2],
                                     func=Act.Abs)
                cosg = work.tile([cnt, 1], f32, tag="cs" + str(g),
                                 name=f"cs{g}")
                nc.scalar.activation(out=cosg[:], in_=dd[:, 3:4],
                                     func=Act.Copy, scale=rec[:])
                state["cos" + str(g)] = cosg

            def emit_mask_cm(g, c3ap):
                g0, cnt = GROUPS[g]
                msk = work.tile([cnt, 1], f32, tag="mk" + str(g),
                                name=f"mk{g}")
                nc.vector.tensor_single_scalar(
                    out=msk[:], in_=c3ap, scalar=float(COUNT * MAXDIS),
                    op=Alu.is_lt)
                nc.scalar.copy(out=cm_all[0:cnt, 3 + g:4 + g], in_=msk[:])
                nc.scalar.activation(out=cm_all[0:cnt, g:g + 1],
                                     in_=state["cos" + str(g)],
                                     func=Act.Copy, scale=msk[:])

            def emit_chain_b(g):
                g0, cnt = GROUPS[g]
                sb = state["sb" + str(g)]
                if g == 2:
                    emit_mask_cm(2, sb[:, 5:6])
                    return
                c0 = 1 + 3 * g
                c3 = work.tile([cnt, 1], f32, tag="c3" + str(g),
                               name=f"c3{g}")
                nc.vector.tensor_reduce(out=c3[:],
                                        in_=pmin[0:cnt, c0:c0 + 3],
                                        axis=X, op=Alu.add)
                nc.scalar.activation(out=c3[:], in_=c3[:],
                                     func=Act.Identity, bias=sb[:, 5:6])
                emit_mask_cm(g, c3[:])

            # softmin exponent decode + 3-way min (DVE small ops)
            def finalize(t):
                eint = work.tile([128, 1], i32, tag="eint")
                ef = work.tile([128, 1], f32, tag="ef")
                nc.vector.tensor_single_scalar(
                    out=eint[:], in_=es[:, t:t + 1].bitcast(i32),
                    scalar=23, op=Alu.arith_shift_right)
                nc.vector.tensor_copy(out=ef[:], in_=eint[:])
                nc.vector.tensor_scalar(
                    out=soft[:, t:t + 1], in0=ef[:],
                    scalar1=float(-TSOFT * LN2),
                    scalar2=float(TSOFT * (127.0 * LN2 + LNC)),
                    op0=Alu.mult, op1=Alu.add)
                nc.vector.tensor_tensor(out=pmin[:, t:t + 1],
                                        in0=pminC[:, t:t + 1],
                                        in1=pminD[:, t:t + 1], op=Alu.min)
                nc.vector.tensor_tensor(out=pmin[:, t:t + 1],
                                        in0=pmin[:, t:t + 1],
                                        in1=soft[:, t:t + 1], op=Alu.min)

            # ---- hot loop: 7 row-tiles x 8 matmuls ------------------------
            TAGS = ("A", "B", "C", "D")
            for t in range(RTILES):
                wtile = kg1[:, 128 * t:128 * (t + 1)]
                for h in range(4):
                    ph = psum.tile([128, 1024], f32, tag=TAGS[h],
                                   name=f"p{TAGS[h]}")
                    for j in range(2):
                        nc.tensor.matmul(
                            ph[:, 512 * j:512 * (j + 1)], wtile,
                            mslice[h][:, 512 * j:512 * (j + 1)],
                            start=True, stop=True)
                    if h == 0:
                        # softmin bank A; Exp frees the bank early so the
                        # selector matmuls can slot into the A ring
                        junkA = work.tile([128, 1024], bf16, tag="jA")
                        eA = work.tile([128, 1], f32, tag="eA")
                        nc.scalar.activation(out=junkA[:], in_=ph[:],
                                             func=Act.Exp,
                                             scale=float(-1.0 / TSOFT),
                                             bias=lnc_sb[:], accum_out=eA[:])
                        if 2 <= t <= 4:
                            emit_sel_mm(t - 2)
                    elif h == 1:
                        junkB = work.tile([128, 1024], bf16, tag="jB")
                        eB = work.tile([128, 1], f32, tag="eB")
                        nc.scalar.activation(out=junkB[:], in_=ph[:],
                                             func=Act.Exp,
                                             scale=float(-1.0 / TSOFT),
                                             bias=lnc_sb[:], accum_out=eB[:])
                        nc.gpsimd.tensor_add(es[:, t:t + 1], eA[:], eB[:])
                    elif h == 2:
                        nc.vector.tensor_reduce(out=pminC[:, t:t + 1],
                                                in_=ph[:], axis=X, op=Alu.min)
                    else:
                        nc.vector.tensor_reduce(out=pminD[:, t:t + 1],
                                                in_=ph[:], axis=X, op=Alu.min)

                if t > 0:
                    finalize(t - 1)
                if t == 1:
                    # endpoint/leftover mins -> uext6 (epilog gather moving)
                    nc.vector.tensor_copy(out=uext6[:, 0:5],
                                          in_=su[:, PAIR2:PAIR2 + 5])
                    nc.vector.tensor_copy(out=uext6[0:SROWS, 5:6],
                                          in_=pmin[0:SROWS, 0:1])
                if t == 3:
                    emit_chain_a(0)
                if t == 4:
                    emit_chain_a(1)
                if t == 5:
                    emit_chain_a(2)
                    emit_chain_b(2)
                    emit_chain_b(0)
                if t == 6:
                    # partial c3 for the last group: cols 4,5 + gathered sums
                    sb1 = state["sb1"]
                    c3p = work.tile([128, 1], f32, tag="c3p")
                    nc.vector.tensor_reduce(out=c3p[:],
                                            in_=pmin[0:128, 4:6],
                                            axis=X, op=Alu.add)
                    nc.scalar.activation(out=c3p[:], in_=c3p[:],
                                         func=Act.Identity, bias=sb1[:, 5:6])
                    state["c3p"] = c3p

            finalize(RTILES - 1)
            c3f = work.tile([128, 1], f32, tag="c3f")
            nc.vector.tensor_tensor(out=c3f[:], in0=state["c3p"][:],
                                    in1=pmin[0:128, 6:7], op=Alu.add)
            emit_mask_cm(1, c3f[:])

            tot = psum.tile([1, 6], f32, tag="A")
            nc.tensor.matmul(tot[:], onescol[:], cm_all[:],
                             start=True, stop=True)
            res = work.tile([1, 2], f32, tag="res")
            tot3 = tot[:].rearrange("p (j c) -> p j c", c=3)
            nc.vector.tensor_reduce(out=res[:], in_=tot3, axis=X, op=Alu.add)
            nc.sync.dma_start(out=out_d.rearrange("(a b) -> a b", a=1),
                              in_=res[:])

    nc.compile()
    return nc


def _make_in_maps(recon_points: np.ndarray, gt_points: np.ndarray):
    s12 = _get_consts()
    recon_points = np.ascontiguousarray(recon_points, np.float32)
    gt_points = np.ascontiguousarray(gt_points, np.float32)

    t_int = np.array([0.25, 0.5, 0.75], np.float32)          # interior t
    pi = np.array([p[0] for p in PAIRS], np.int32)
    pj = np.array([p[1] for p in PAIRS], np.int32)

    in_maps = []
    for k in range(N_CORES):
        rec = recon_points[BPC * k:BPC * (k + 1)]            # [2, 17, 2]
        # query points K for every row of the permuted layout
        K = np.zeros((RPAD, D), np.float32)
        blk = np.zeros((RPAD, BPC), np.float32)
        for b in range(BPC):
            P1 = rec[b]                                      # [17, 2]
            K[N * b:N * b + N] = P1                          # endpoints
            blk[N * b:N * b + N, b] = 1.0
            # interiors of pairs 0..127 -> rtiles 1-3 (b0) / 4-6 (b1)
            seg = (t_int[:, None, None] * P1[pi[:128]][None]
                   + (1.0 - t_int)[:, None, None] * P1[pj[:128]][None])
            base = 128 * (1 + 3 * b)
            K[base:base + 384] = seg.reshape(384, 2)
            blk[base:base + 384, b] = 1.0
            # leftover pair interiors -> rows 34..81 of rtile 0
            segl = (t_int[None, :, None] * P1[pi[128:]][:, None]
                    + (1.0 - t_int)[None, :, None] * P1[pj[128:]][:, None])
            lbase = 2 * N + 3 * NLEFT * b
            K[lbase:lbase + 3 * NLEFT] = segl.reshape(3 * NLEFT, 2)
            blk[lbase:lbase + 3 * NLEFT, b] = 1.0
        k2 = (K * K).sum(-1) + 0.25
        kaugT = np.zeros((CONTR, RPAD), np.float32)
        kaugT[0] = k2 * blk[:, 0]
        kaugT[1] = k2 * blk[:, 1]
        for b in range(BPC):
            kaugT[2 + 3 * b] = -2.0 * K[:, 0] * blk[:, b]
            kaugT[3 + 3 * b] = -2.0 * K[:, 1] * blk[:, b]
            kaugT[4 + 3 * b] = blk[:, b]

        gt_pair = gt_points[BPC * k:BPC * (k + 1)]           # [2, 4096, 2]
        gtt = np.empty((CONTR, M), np.float32)
        gtt[0:2] = 1.0
        gtt[2:4] = gt_pair[0].T
        gtt[4] = (gt_pair[0] * gt_pair[0]).sum(-1) - 0.25
        gtt[5:7] = gt_pair[1].T
        gtt[7] = (gt_pair[1] * gt_pair[1]).sum(-1) - 0.25

        uex = np.zeros((SROWS, 5), np.float32)
        for b in range(BPC):
            u = rec[b, 0:1] - rec[b]                         # [17, 2]
            u2 = (u * u).sum(-1)
            uex[N * b:N * b + N, 0:2] = u
            uex[N * b:N * b + N, 2] = u2
            uex[N * b:N * b + N, 4] = u2 + D * EPS_ABS
            uex[N * b:N * b + N, 3] = np.sqrt(u2 + D * EPS_ABS)

        kg1 = np.concatenate([kaugT, gtt[:, :M // 2]], 1)
        kg2 = np.ascontiguousarray(gtt[:, M // 2:])
        su = np.concatenate([s12, uex], 1)
        in_maps.append({"kg1": kg1, "kg2": kg2, "su": su})
    return in_maps


def kernel(recon_points: np.ndarray, gt_points: np.ndarray) -> np.ndarray:
    from concourse.bass_utils import run_bass_kernel_spmd

    global _COMPILED
    if _COMPILED is None:
        _COMPILED = _build()
    nc = _COMPILED

    in_maps = _make_in_maps(recon_points, gt_points)
    res = run_bass_kernel_spmd(nc, in_maps, core_ids=list(range(N_CORES)))
    partials = np.stack([r["out"] for r in res.results])  # [8, 2]
    cos_sum = partials[:, 0].sum(dtype=np.float32)
    cnt = partials[:, 1].sum(dtype=np.float32)
    return np.float32(cos_sum / (np.float32(1.0) + cnt))


# revision 36
# speedup vs baseline: 1.2130x; 1.0750x over previous
"""Trainium2 Bass kernel for nn_ComputeVecLoss (vector loss over keypoint graphs).

Math (per batch b):
  For every keypoint pair (i>j) sample 5 points on the segment; cdis = mean
  over the 5 points of the min squared distance to the 4096 gt points; an edge
  exists when cdis < 1e-3.  Loss = sum over edges of |u_i.u_j| / (|u_i||u_j|)
  divided by (1 + edge count), u_k = p0 - p_k.

Design:
  * Each batch needs only 425 unique query points (17 endpoints + 136*3
    interiors).  Two batches per core -> 850 rows in 7 tiles of 128.
  * d2(r,m) = |K_r|^2 + |g_m|^2 - 2 K_r.g_m is ONE TensorEngine contraction
    of depth 8 handling both batches: kaugT rows [k2_b0, k2_b1, -2Kx0, -2Ky0,
    blk0, -2Kx1, -2Ky1, blk1] vs gaug rows [1, 1, gx0, gy0, g2_0-1/4, gx1,
    gy1, g2_1-1/4].  kaugT ([8,896]) is built on HOST and ships in the same
    DMA as the first half of the gt rows, so the PE starts ~3us in.
  * min over m=4096 per row-tile: banks A,B (m<2048) via exp-softmin on the
    Scalar engine (ACT Exp with accumulate - frees their PSUM banks early),
    banks C,D hard-min tensor_reduce on Vector.  The softmin exponent decode
    and the 3-way min run on the otherwise-idle GpSimd engine.
  * Row-tile 0 holds the 34 endpoints + 48 leftover-pair interiors, so the
    cosine epilog (selector gathers + chains) runs DURING the hot loop;
    only the last group's threshold test and the final sum trail the loop.
  * Sum-gather: s12 = s1+s2 gathers u_i+u_j / |u_i|^2+|u_j|^2 / absu_i+absu_j
    per pair in ONE matmul per group; dot and |u_i||u_j| come back via
    difference of squares (the 1/2 factors cancel in the ratio).

Sharding: batch dim 16 -> 8 cores x 2 batches.  Each core returns
[sum(cos), edge_count]; the host combines and divides.
"""

import os
import sys

for _p in ("/opt/trn_rl_repo",):
    if os.path.isdir(_p) and _p not in sys.path:
        sys.path.append(_p)

import numpy as np

B, N, D = 16, 17, 2
M = 4096
COUNT = 5
MAXDIS = 1e-3
EPS_ABS = 1e-5
N_CORES = 8
TSOFT = 8e-5           # softmin temperature
LNC = 34.657359028     # ln(2^50) prescale keeps es well inside fp32 normals
LN2 = 0.6931471805599453
BPC = B // N_CORES          # batches per core
NPAIR = N * (N - 1) // 2    # 136
RTILES = 7
RPAD = RTILES * 128         # 896
CONTR = 8                   # contraction depth of the hot matmul
PAIR2 = BPC * NPAIR         # 272 pairs per core
NLEFT = NPAIR - 128         # 8 leftover pairs per batch
SROWS = 2 * N + 2 * 3 * NLEFT  # 82 rows: 34 endpoints + 48 leftover triples
GROUPS = [(0, 128), (128, 128), (256, 2 * NLEFT)]
SUW = PAIR2 + 5             # s12 + uex columns

PAIRS = [(i, j) for i in range(1, N) for j in range(i)]


def _row_endpoint(b, i):
    return N * b + i


def _row_leftover(b, q, k):
    return 2 * N + 3 * (NLEFT * b + q) + k


def _s12_const():
    s = np.zeros((SROWS, PAIR2), np.float32)
    for b in range(BPC):
        for p, (i, j) in enumerate(PAIRS):
            if p < 128:
                P = 128 * b + p
            else:
                P = 256 + NLEFT * b + (p - 128)
                for k in range(3):
                    s[_row_leftover(b, p - 128, k), P] = 1.0
            s[_row_endpoint(b, i), P] += 1.0
            s[_row_endpoint(b, j), P] += 1.0
    return s


_CONSTS = None
_COMPILED = None


def _get_consts():
    global _CONSTS
    if _CONSTS is None:
        _CONSTS = _s12_const()
    return _CONSTS


def _build():
    import concourse.bass as bass
    import concourse.bacc as bacc
    import concourse.tile as tile
    from concourse import mybir

    f32 = mybir.dt.float32
    f32r = mybir.dt.float32r
    bf16 = mybir.dt.bfloat16
    i32 = mybir.dt.int32
    Alu = mybir.AluOpType
    Act = mybir.ActivationFunctionType
    X = mybir.AxisListType.X

    nc = bacc.Bacc("TRN2", target_bir_lowering=False, debug=False,
                   num_devices=N_CORES)

    # gt rows land as four 8-row bands at partition bases 0/32/64/96 of a
    # [128, 1024] tile: DMA-to-SBUF bandwidth scales with the partition
    # count, so writing 32 partitions loads ~4x faster than the natural
    # [8, wide] layout.  The PE reads each band via an explicit
    # tile_position=(0,0) (moving-operand bases must be 32-aligned).
    kaug_d = nc.dram_tensor("kaugB", [96, RPAD], f32r,
                            kind="ExternalInput").ap()
    gtb_d = nc.dram_tensor("gtb", [96, 1024], f32r,
                           kind="ExternalInput").ap()
    gtb2_d = nc.dram_tensor("gtb2", [CONTR, 1024], f32r,
                            kind="ExternalInput").ap()
    su_d = nc.dram_tensor("su", [SROWS, SUW], f32, kind="ExternalInput").ap()
    out_d = nc.dram_tensor("out", [2], f32, kind="ExternalOutput").ap()

    with tile.TileContext(nc) as tc:
        with (
            tc.tile_pool(name="singles", bufs=1) as singles,
            tc.tile_pool(name="work", bufs=2) as work,
            tc.tile_pool(name="psum", bufs=1, space="PSUM") as psum,
        ):
            # ---- inputs: gt + epilog constants on sync, weights on scalar -
            kaugB = singles.tile([96, RPAD], f32r)
            gtb = singles.tile([96, 1024], f32r)
            gtb2 = singles.tile([CONTR, 1024], f32r)
            su = singles.tile([SROWS, SUW], f32)
            nc.sync.dma_start(out=gtb[:], in_=gtb_d[:])
            nc.scalar.dma_start(out=kaugB[:], in_=kaug_d[:])
            nc.scalar.dma_start(out=gtb2[:], in_=gtb2_d[:])
            nc.sync.dma_start(out=su[:], in_=su_d[:])
            s12_sb = su[:, 0:PAIR2]

            # ---- small setup: memsets on gpsimd, ACT table warm -----------
            onescol = singles.tile([128, 1], f32)
            nc.gpsimd.memset(onescol[:], 1.0)
            cm_all = singles.tile([128, 6], f32)
            nc.gpsimd.memset(cm_all[:], 0.0)
            lnc_sb = singles.tile([128, 1], f32)
            nc.gpsimd.memset(lnc_sb[:], float(LNC))
            warm = singles.tile([1, 2], f32)
            nc.gpsimd.memset(warm[:], 1.0)
            nc.scalar.activation(out=warm[:, 0:1], in_=warm[:, 1:2],
                                 func=Act.Square)
            nc.scalar.activation(out=warm[:, 0:1], in_=warm[:, 1:2],
                                 func=Act.Abs)
            nc.scalar.activation(out=warm[:, 0:1], in_=warm[:, 1:2],
                                 func=Act.Exp)

            uext6 = singles.tile([SROWS, 6], f32)
            pminC = singles.tile([128, RTILES], f32)
            pminD = singles.tile([128, RTILES], f32)
            pmin = singles.tile([128, RTILES], f32)
            es = singles.tile([128, RTILES], f32)
            soft = singles.tile([128, RTILES], f32)

            state = {}

            def emit_sel_mm(g):
                g0, cnt = GROUPS[g]
                pg = psum.tile([cnt, 6], f32, tag="A", name=f"pg{g}")
                nc.tensor.matmul(pg[:], s12_sb[:, g0:g0 + cnt],
                                 uext6[:], start=True, stop=True)
                sb = work.tile([cnt, 6], f32, tag="sb" + str(g),
                               name=f"sbg{g}")
                nc.scalar.copy(out=sb[:], in_=pg[:])
                state["sb" + str(g)] = sb

            def emit_chain_a(g):
                # cos = |u_i.u_j| / (|u_i||u_j|) via difference of squares;
                # the elementwise math runs on the idle GpSimd engine
                g0, cnt = GROUPS[g]
                sb = state["sb" + str(g)]
                sq = work.tile([cnt, 3], f32, tag="sq" + str(g),
                               name=f"sq{g}")
                dd = work.tile([cnt, 4], f32, tag="dd" + str(g),
                               name=f"dd{g}")
                nc.gpsimd.tensor_mul(sq[:, 0:2], sb[:, 0:2], sb[:, 0:2])
                nc.gpsimd.tensor_tensor(out=dd[:, 0:1], in0=sq[:, 0:1],
                                        in1=sq[:, 1:2], op=Alu.add)
                nc.gpsimd.tensor_mul(sq[:, 2:3], sb[:, 3:4], sb[:, 3:4])
                nc.gpsimd.tensor_tensor(out=dd[:, 1:2], in0=dd[:, 0:1],
                                        in1=sb[:, 2:3], op=Alu.subtract)
                nc.gpsimd.tensor_tensor(out=dd[:, 2:3], in0=sq[:, 2:3],
                                        in1=sb[:, 4:5], op=Alu.subtract)
                rec = work.tile([cnt, 1], f32, tag="rc" + str(g),
                                name=f"rc{g}")
                nc.vector.reciprocal(out=rec[:], in_=dd[:, 2:3])
                nc.scalar.activation(out=dd[:, 3:4], in_=dd[:, 1:2],
                                     func=Act.Abs)
                cosg = work.tile([cnt, 1], f32, tag="cs" + str(g),
                                 name=f"cs{g}")
                nc.gpsimd.tensor_mul(cosg[:], dd[:, 3:4], rec[:])
                state["cos" + str(g)] = cosg

            def emit_mask_cm(g, c3ap):
                g0, cnt = GROUPS[g]
                msk = work.tile([cnt, 1], f32, tag="mk" + str(g),
                                name=f"mk{g}")
                nc.vector.tensor_single_scalar(
                    out=msk[:], in_=c3ap, scalar=float(COUNT * MAXDIS),
                    op=Alu.is_lt)
                nc.scalar.copy(out=cm_all[0:cnt, 3 + g:4 + g], in_=msk[:])
                nc.scalar.activation(out=cm_all[0:cnt, g:g + 1],
                                     in_=state["cos" + str(g)],
                                     func=Act.Copy, scale=msk[:])

            def emit_chain_b(g):
                g0, cnt = GROUPS[g]
                sb = state["sb" + str(g)]
                if g == 2:
                    emit_mask_cm(2, sb[:, 5:6])
                    return
                c0 = 1 + 3 * g
                c3 = work.tile([cnt, 1], f32, tag="c3" + str(g),
                               name=f"c3{g}")
                nc.vector.tensor_reduce(out=c3[:],
                                        in_=pmin[0:cnt, c0:c0 + 3],
                                        axis=X, op=Alu.add)
                nc.gpsimd.tensor_tensor(out=c3[:], in0=c3[:],
                                        in1=sb[:, 5:6], op=Alu.add)
                emit_mask_cm(g, c3[:])

            # softmin exponent decode + 3-way min (DVE small ops)
            def finalize(t):
                eint = work.tile([128, 1], i32, tag="eint")
                ef = work.tile([128, 1], f32, tag="ef")
                nc.vector.tensor_single_scalar(
                    out=eint[:], in_=es[:, t:t + 1].bitcast(i32),
                    scalar=23, op=Alu.arith_shift_right)
                nc.vector.tensor_copy(out=ef[:], in_=eint[:])
                nc.vector.tensor_scalar(
                    out=soft[:, t:t + 1], in0=ef[:],
                    scalar1=float(-TSOFT * LN2),
                    scalar2=float(TSOFT * (127.0 * LN2 + LNC)),
                    op0=Alu.mult, op1=Alu.add)
                nc.vector.tensor_tensor(out=pmin[:, t:t + 1],
                                        in0=pminC[:, t:t + 1],
                                        in1=pminD[:, t:t + 1], op=Alu.min)
                nc.vector.tensor_tensor(out=pmin[:, t:t + 1],
                                        in0=pmin[:, t:t + 1],
                                        in1=soft[:, t:t + 1], op=Alu.min)

            # ---- hot loop: 7 row-tiles x 8 matmuls ------------------------
            TAGS = ("A", "B", "C", "D")
            for t in range(RTILES):
                for h in range(4):
                    hb = 32 * h if h < 3 else 0
                    wtile = kaugB[hb:hb + CONTR, 128 * t:128 * (t + 1)]
                    fm = (gtb[hb:hb + CONTR, :] if h < 3 else gtb2[:])
                    ph = psum.tile([128, 1024], f32, tag=TAGS[h],
                                   name=f"p{TAGS[h]}")
                    for j in range(2):
                        nc.tensor.matmul(
                            ph[:, 512 * j:512 * (j + 1)], wtile,
                            fm[:, 512 * j:512 * (j + 1)],
                            start=True, stop=True)
                    if h == 0:
                        # softmin bank A; Exp frees the bank early so the
                        # selector matmuls can slot into the A ring
                        junkA = work.tile([128, 1024], bf16, tag="jA")
                        eA = work.tile([128, 1], f32, tag="eA")
                        nc.scalar.activation(out=junkA[:], in_=ph[:],
                                             func=Act.Exp,
                                             scale=float(-1.0 / TSOFT),
                                             bias=lnc_sb[:], accum_out=eA[:])
                        if 2 <= t <= 4:
                            emit_sel_mm(t - 2)
                    elif h == 1:
                        junkB = work.tile([128, 1024], bf16, tag="jB")
                        eB = work.tile([128, 1], f32, tag="eB")
                        nc.scalar.activation(out=junkB[:], in_=ph[:],
                                             func=Act.Exp,
                                             scale=float(-1.0 / TSOFT),
                                             bias=lnc_sb[:], accum_out=eB[:])
                        nc.gpsimd.tensor_add(es[:, t:t + 1], eA[:], eB[:])
                    elif h == 2:
                        nc.vector.tensor_reduce(out=pminC[:, t:t + 1],
                                                in_=ph[:], axis=X, op=Alu.min)
                    else:
                        nc.vector.tensor_reduce(out=pminD[:, t:t + 1],
                                                in_=ph[:], axis=X, op=Alu.min)

                if t > 0:
                    finalize(t - 1)
                if t == 1:
                    # endpoint/leftover mins -> uext6 (epilog gather moving)
                    nc.vector.tensor_copy(out=uext6[:, 0:5],
                                          in_=su[:, PAIR2:PAIR2 + 5])
                    nc.vector.tensor_copy(out=uext6[0:SROWS, 5:6],
                                          in_=pmin[0:SROWS, 0:1])
                if t == 3:
                    emit_chain_a(0)
                if t == 4:
                    emit_chain_a(1)
                if t == 5:
                    emit_chain_a(2)
                    emit_chain_b(2)
                    emit_chain_b(0)
                if t == 6:
                    # partial c3 for the last group: cols 4,5 + gathered sums
                    sb1 = state["sb1"]
                    c3p = work.tile([128, 1], f32, tag="c3p")
                    nc.vector.tensor_reduce(out=c3p[:],
                                            in_=pmin[0:128, 4:6],
                                            axis=X, op=Alu.add)
                    nc.gpsimd.tensor_tensor(out=c3p[:], in0=c3p[:],
                                            in1=sb1[:, 5:6], op=Alu.add)
                    state["c3p"] = c3p

            finalize(RTILES - 1)
            # rides the gpsimd queue right behind finalize's mins
            c3f = work.tile([128, 1], f32, tag="c3f")
            nc.gpsimd.tensor_tensor(out=c3f[:], in0=state["c3p"][:],
                                    in1=pmin[0:128, 6:7], op=Alu.add)
            nc.vector.tensor_single_scalar(
                out=cm_all[0:128, 4:5], in_=c3f[:],
                scalar=float(COUNT * MAXDIS), op=Alu.is_lt)
            nc.scalar.activation(out=cm_all[0:128, 1:2],
                                 in_=state["cos1"], func=Act.Copy,
                                 scale=cm_all[0:128, 4:5])

            tot = psum.tile([1, 6], f32, tag="A")
            nc.tensor.matmul(tot[:], onescol[:], cm_all[:],
                             start=True, stop=True)
            res = work.tile([1, 2], f32, tag="res")
            tot3 = tot[:].rearrange("p (j c) -> p j c", c=3)
            nc.vector.tensor_reduce(out=res[:], in_=tot3, axis=X, op=Alu.add)
            nc.sync.dma_start(out=out_d.rearrange("(a b) -> a b", a=1),
                              in_=res[:])

    nc.compile()
    return nc


def _make_in_maps(recon_points: np.ndarray, gt_points: np.ndarray):
    s12 = _get_consts()
    recon_points = np.ascontiguousarray(recon_points, np.float32)
    gt_points = np.ascontiguousarray(gt_points, np.float32)

    t_int = np.array([0.25, 0.5, 0.75], np.float32)          # interior t
    pi = np.array([p[0] for p in PAIRS], np.int32)
    pj = np.array([p[1] for p in PAIRS], np.int32)

    in_maps = []
    for k in range(N_CORES):
        rec = recon_points[BPC * k:BPC * (k + 1)]            # [2, 17, 2]
        # query points K for every row of the permuted layout
        K = np.zeros((RPAD, D), np.float32)
        blk = np.zeros((RPAD, BPC), np.float32)
        for b in range(BPC):
            P1 = rec[b]                                      # [17, 2]
            K[N * b:N * b + N] = P1                          # endpoints
            blk[N * b:N * b + N, b] = 1.0
            # interiors of pairs 0..127 -> rtiles 1-3 (b0) / 4-6 (b1)
            seg = (t_int[:, None, None] * P1[pi[:128]][None]
                   + (1.0 - t_int)[:, None, None] * P1[pj[:128]][None])
            base = 128 * (1 + 3 * b)
            K[base:base + 384] = seg.reshape(384, 2)
            blk[base:base + 384, b] = 1.0
            # leftover pair interiors -> rows 34..81 of rtile 0
            segl = (t_int[None, :, None] * P1[pi[128:]][:, None]
                    + (1.0 - t_int)[None, :, None] * P1[pj[128:]][:, None])
            lbase = 2 * N + 3 * NLEFT * b
            K[lbase:lbase + 3 * NLEFT] = segl.reshape(3 * NLEFT, 2)
            blk[lbase:lbase + 3 * NLEFT, b] = 1.0
        k2 = (K * K).sum(-1) + 0.25
        kaugT = np.zeros((CONTR, RPAD), np.float32)
        kaugT[0] = k2 * blk[:, 0]
        kaugT[1] = k2 * blk[:, 1]
        for b in range(BPC):
            kaugT[2 + 3 * b] = -2.0 * K[:, 0] * blk[:, b]
            kaugT[3 + 3 * b] = -2.0 * K[:, 1] * blk[:, b]
            kaugT[4 + 3 * b] = blk[:, b]

        gt_pair = gt_points[BPC * k:BPC * (k + 1)]           # [2, 4096, 2]
        gtt = np.empty((CONTR, M), np.float32)
        gtt[0:2] = 1.0
        gtt[2:4] = gt_pair[0].T
        gtt[4] = (gt_pair[0] * gt_pair[0]).sum(-1) - 0.25
        gtt[5:7] = gt_pair[1].T
        gtt[7] = (gt_pair[1] * gt_pair[1]).sum(-1) - 0.25

        uex = np.zeros((SROWS, 5), np.float32)
        for b in range(BPC):
            u = rec[b, 0:1] - rec[b]                         # [17, 2]
            u2 = (u * u).sum(-1)
            uex[N * b:N * b + N, 0:2] = u
            uex[N * b:N * b + N, 2] = u2
            uex[N * b:N * b + N, 4] = u2 + D * EPS_ABS
            uex[N * b:N * b + N, 3] = np.sqrt(u2 + D * EPS_ABS)

        # banded DMA layout for gt: 8-row chunks at partitions 0/32/64
        # (zero-padded, plain full-tile transfer; chunk 3 ships naturally);
        # weights replicated so fmap and weights share a partition base
        gtb = np.zeros((96, 1024), np.float32)
        kaugB = np.zeros((96, RPAD), np.float32)
        for c in range(3):
            gtb[32 * c:32 * c + CONTR] = gtt[:, 1024 * c:1024 * (c + 1)]
            kaugB[32 * c:32 * c + CONTR] = kaugT
        gtb2 = np.ascontiguousarray(gtt[:, 3072:4096])
        su = np.concatenate([s12, uex], 1)
        in_maps.append({"kaugB": kaugB, "gtb": gtb, "gtb2": gtb2, "su": su})
    return in_maps


def kernel(recon_points: np.ndarray, gt_points: np.ndarray) -> np.ndarray:
    from concourse.bass_utils import run_bass_kernel_spmd

    global _COMPILED
    if _COMPILED is None:
        _COMPILED = _build()
    nc = _COMPILED

    in_maps = _make_in_maps(recon_points, gt_points)
    res = run_bass_kernel_spmd(nc, in_maps, core_ids=list(range(N_CORES)))
    partials = np.stack([r["out"] for r in res.results])  # [8, 2]
    cos_sum = partials[:, 0].sum(dtype=np.float32)
    cnt = partials[:, 1].sum(dtype=np.float32)
    return np.float32(cos_sum / (np.float32(1.0) + cnt))
